# revision 16
# baseline (speedup 1.0000x reference)
"""Causal multi-head attention block (dense transformer) on 8 Trainium2 cores.

Reference computation (fp32):
    qkv = res @ W_attn.T + b_attn            # [B,S,3D]
    q,k,v -> heads [B,H,S,Dh];  scores = q k^T / sqrt(Dh), causal softmax
    inter = probs @ v -> [B,S,D];  out = inter @ W_O.T + b_O

Sharding: core (b, hg) handles batch b and head-group hg (4 heads).
Each core computes its 4 heads' attention and a partial output
projection over its 256 inter dims; the host sums the 4 partials per
batch and adds b_O.

Device dataflow (per core) keeps activations feature-on-partition
("T layout") so every matmul contraction is on the partition dim:
    resT [D,S]  --W_qkv-->  qT,kT [256,S] and v [S,256] (natural)
    S^T tile = kT_h.T-slices @ qT_h   (K=Dh=64, packed 2 heads via
    tile_position row groups)
    P^T = exp(S^T) (ACT, scale folded into q), causal handled by
    skipping blocks above the diagonal, column-sliced partial tiles,
    and a 0/1 triangular mask multiply on diagonal 128x128 blocks.
    PV: lhsT = [v_h | ones64] so PSUM rows 0-63 get inter^T_h and rows
    64-127 get the softmax denominator; normalize with DVE reciprocal +
    multiply straight out of PSUM.
    out_partial = interT.T @ W_O^T-slice  -> DMA out.

All matmul operands are float32r (verified ~1.6e-4 rel err, full PE rate).
"""

import numpy as np

import concourse.bass as bass
import concourse.mybir as mybir
import concourse.tile as tile
from concourse.bass_utils import run_bass_kernel_spmd

F32 = mybir.dt.float32
F32R = mybir.dt.float32r
AF = mybir.ActivationFunctionType

# Problem constants (hardcoded per contract)
B = 2
S = 2048
D = 1024
H = 16
DH = 64
N_CORES = 8
HG = 4            # head groups (cores per batch)
NH = H // HG      # heads per core = 4
FH = NH * DH      # features per core per q/k/v = 256
DT = D // 128     # d-model tiles = 8
NT = S // 128     # sequence j-tiles = 16
SC = 512          # i-chunk width
NC_CHUNK = S // SC  # 4


def _split_multi_waits(nc, max_waits=1):
    """walrus in this container rejects >1 sync-wait per instruction; hoist
    extras onto preceding NoOps on the same engine (queue order preserves
    semantics)."""
    for fn in nc.m.functions:
        for blk in fn.blocks:
            insts = list(blk.instructions)
            out = []
            changed = False
            for inst in insts:
                si = inst.sync_info
                waits = list(si.on_wait) if (si is not None and si.on_wait) else []
                if len(waits) > max_waits:
                    extra, keep = waits[:-max_waits], waits[-max_waits:]
                    for j, w in enumerate(extra):
                        out.append(mybir.InstNoOp(
                            name=f"{inst.name}-ws{j}", ins=[], outs=[],
                            engine=inst.engine,
                            sync_info=mybir.SyncInfo(on_wait=[w], on_update=[]),
                        ))
                    inst.sync_info = mybir.SyncInfo(
                        on_wait=keep, on_update=list(si.on_update))
                    changed = True
                out.append(inst)
            if changed:
                blk.instructions = out


def build_nc(niter=1, io_mode="external"):
    import contextlib
    nc = bass.Bass()

    if io_mode == "external":
        resT_d = nc.dram_tensor("resT", [D, S], F32R, kind="ExternalInput")
        wqkv_d = nc.dram_tensor("wqkvT", [D, 3 * FH], F32R, kind="ExternalInput")
        bqk_d = nc.dram_tensor("bqk", [4, 128], F32, kind="ExternalInput")
        bv_d = nc.dram_tensor("bv", [FH], F32, kind="ExternalInput")
        woT_d = nc.dram_tensor("woT", [FH, D], F32R, kind="ExternalInput")
        part_d = nc.dram_tensor("part", [S, D], F32, kind="ExternalOutput")
    else:
        # timing-only build: no big host<->device transfers
        resT_d = nc.dram_tensor("resT", [D, S], F32R)
        wqkv_d = nc.dram_tensor("wqkvT", [D, 3 * FH], F32R)
        bqk_d = nc.dram_tensor("bqk", [4, 128], F32)
        bv_d = nc.dram_tensor("bv", [FH], F32)
        woT_d = nc.dram_tensor("woT", [FH, D], F32R)
        part_d = nc.dram_tensor("part", [S, D], F32)
        tick_d = nc.dram_tensor("tick", [128, 1], F32, kind="ExternalInput")
        tock_d = nc.dram_tensor("tock", [128, 1], F32, kind="ExternalOutput")

    with tile.TileContext(nc) as tc:
        with (
            tc.tile_pool(name="persist", bufs=1) as persist,
            tc.tile_pool(name="work", bufs=3) as work,
            tc.tile_pool(name="expp", bufs=1) as expp,
            tc.tile_pool(name="rdenp", bufs=2) as rdenp,
            tc.tile_pool(name="outp", bufs=2) as outp,
            tc.tile_pool(name="ps_mm", bufs=3, space="PSUM") as ps_mm,
        ):
          with (tc.For_i(0, niter, 1) if niter > 1
                else contextlib.nullcontext()):
            if io_mode != "external":
                tick_sb = persist.tile([128, 1], F32, tag="tick")
                nc.sync.dma_start(tick_sb, tick_d[:, :])
                nc.sync.dma_start(tock_d[:, :], tick_sb)
            # ---- constant / persistent loads ----
            w_sb = persist.tile([128, DT, 3 * FH], F32R, tag="w")
            nc.sync.dma_start(
                w_sb, wqkv_d[:, :].rearrange("(t p) f -> p t f", p=128))
            woT_sb = persist.tile([128, 2, D], F32R, tag="wo")
            nc.sync.dma_start(
                woT_sb, woT_d[:, :].rearrange("(t p) f -> p t f", p=128))
            bias_sb = persist.tile([128, 4], F32, tag="bqk")
            nc.sync.dma_start(
                bias_sb, bqk_d[:, :].rearrange("f p -> p f"))
            vb_sb = persist.tile([128, FH], F32, tag="bv")
            bv_ap = bv_d[:]
            nc.gpsimd.dma_start(
                vb_sb,
                bass.AP(tensor=bv_ap.tensor, offset=bv_ap.offset,
                        ap=[[0, 128], [1, FH]]))
            # 0/1 upper-triangular (keep i>=j) mask for diagonal blocks
            tri01 = persist.tile([128, 128], F32, tag="tri")
            nc.gpsimd.memset(tri01, 0.0)
            nc.gpsimd.affine_select(
                out=tri01, in_=tri01,
                compare_op=mybir.AluOpType.is_gt,
                fill=1.0, base=0, pattern=[[-1, 128]], channel_multiplier=1)

            resT_sb = []
            for dt_i in range(DT):
                t = persist.tile([128, S], F32R, tag=f"resT{dt_i}")
                nc.sync.dma_start(t, resT_d[128 * dt_i:128 * (dt_i + 1), :])
                resT_sb.append(t)

            # ---- phase A: qkv projections ----
            # qk tiles: ft in {0:q01, 1:q23, 2:k01, 3:k23} x chunk c -> [128, SC]
            qk_sb = {}
            v_sb = []
            for c in range(NC_CHUNK):
                for ft in range(4):
                    qk = persist.tile([128, SC], F32R, tag=f"qk{ft}_{c}")
                    ps = ps_mm.tile([128, SC], F32, tag="mm", name=f"psqk{ft}_{c}", padded_shape=[128, SC])
                    wcol = 128 * ft if ft < 2 else FH + 128 * (ft - 2)
                    for dt_i in range(DT):
                        nc.tensor.matmul(
                            ps, w_sb[:, dt_i, wcol:wcol + 128],
                            resT_sb[dt_i][:, SC * c:SC * (c + 1)],
                            start=(dt_i == 0), stop=(dt_i == DT - 1))
                    nc.scalar.activation(qk, ps, AF.Identity,
                                         bias=bias_sb[:, ft:ft + 1], scale=1.0)
                    qk_sb[(ft, c)] = qk
                # v s-tiles for this chunk: st = 4c..4c+3, layout per head
                # [v_h(64) | ones(64)] -> [128, NH*128]
                for st in range(4 * c, 4 * c + 4):
                    vt = persist.tile([128, NH, 128], F32R, tag=f"v{st}")
                    for h in range(NH):
                        nc.vector.memset(vt[:, h, DH:128].bitcast(F32), 1.0)
                    psv = ps_mm.tile([128, FH], F32, tag="mm", name=f"psv{st}", padded_shape=[128, SC])
                    for dt_i in range(DT):
                        nc.tensor.matmul(
                            psv, resT_sb[dt_i][:, 128 * st:128 * (st + 1)],
                            w_sb[:, dt_i, 2 * FH:3 * FH],
                            start=(dt_i == 0), stop=(dt_i == DT - 1))
                    nc.vector.tensor_tensor(
                        vt[:, :, 0:DH],
                        psv.rearrange("p (h d) -> p h d", d=DH),
                        vb_sb.rearrange("p (h d) -> p h d", d=DH),
                        mybir.AluOpType.add)
                    v_sb.append(vt)

            # ---- phase B: attention per chunk ----
            interT = {}  # (kt, c) -> [128, SC] f32r
            with (
                tc.tile_pool(name="ps_pv", bufs=1, space="PSUM") as ps_pv,
            ):
                for c in range(NC_CHUNK):
                    pv = ps_pv.tile([128, NH, SC], F32, tag="pv")
                    for h in range(NH):
                        kt_idx, par = divmod(h, 2)
                        base = 64 * par
                        ntile = 4 * c + 4
                        expP = expp.tile([128, 4, SC], F32R, tag="expP")
                        for t in range(ntile):
                            partial = t >= 4 * c
                            dpos = t - 4 * c
                            lo = 128 * dpos if partial else 0
                            sps = ps_mm.tile([128, SC], F32, tag="mm",
                                             name=f"sps{c}_{h}_{t}")
                            kt_tile = qk_sb[(2 + kt_idx, t // 4)]
                            q_tile = qk_sb[(kt_idx, c)]
                            nc.tensor.matmul(
                                sps[:, lo:SC],
                                kt_tile[base:base + 64,
                                        128 * (t % 4):128 * (t % 4) + 128],
                                q_tile[base:base + 64, lo:SC],
                                start=True, stop=True,
                                tile_position=(base, 0))
                            if partial:
                                nc.scalar.activation(
                                    expP[:, dpos, lo:SC], sps[:, lo:SC], AF.Exp)
                            else:
                                expF = work.tile([128, SC], F32R, tag="expF")
                                nc.scalar.activation(expF, sps, AF.Exp)
                                nc.tensor.matmul(
                                    pv[:, h, :], v_sb[t][:, h, :], expF,
                                    start=(t == 0), stop=False)
                        # mask diagonal 128-blocks of the 4 partial tiles
                        for dpos in range(4):
                            nc.vector.tensor_tensor(
                                expP[:, dpos, 128 * dpos:128 * dpos + 128],
                                expP[:, dpos, 128 * dpos:128 * dpos + 128],
                                tri01, mybir.AluOpType.mult)
                        for dpos in range(4):
                            t = 4 * c + dpos
                            lo = 128 * dpos
                            nc.tensor.matmul(
                                pv[:, h, lo:SC], v_sb[t][:, h, :],
                                expP[:, dpos, lo:SC],
                                start=(t == 0), stop=(dpos == 3))
                    # normalize: rows 64-127 of each pv bank hold the denom
                    for h in range(NH):
                        kt_idx, par = divmod(h, 2)
                        key = (kt_idx, c)
                        if key not in interT:
                            interT[key] = persist.tile(
                                [128, SC], F32R, tag=f"it{kt_idx}_{c}",
                                name=f"interT{kt_idx}_{c}")
                        rden = rdenp.tile([128, SC], F32, tag="rden",
                                          name=f"rden{c}_{h}")
                        nc.vector.reciprocal(rden[64:128, :], pv[64:128, h, :])
                        nc.vector.tensor_tensor(
                            interT[key][64 * par:64 * par + 64, :],
                            pv[0:64, h, :], rden[64:128, :],
                            mybir.AluOpType.mult)

            # ---- phase C: output projection ----
            if True:
                for st in range(NT):
                    ot = outp.tile([128, D], F32, tag="ot")
                    for dc in range(2):
                        po = ps_mm.tile([128, 512], F32, tag="mm",
                                        name=f"po{st}_{dc}")
                        for kt in range(2):
                            nc.tensor.matmul(
                                po,
                                interT[(kt, st // 4)][:, 128 * (st % 4):
                                                      128 * (st % 4) + 128],
                                woT_sb[:, kt, 512 * dc:512 * (dc + 1)],
                                start=(kt == 0), stop=(kt == 1))
                        nc.vector.tensor_copy(ot[:, 512 * dc:512 * (dc + 1)], po)
                    nc.sync.dma_start(part_d[128 * st:128 * (st + 1), :], ot)

    _split_multi_waits(nc)
    return nc


_NC_CACHE = None


def kernel(res, W_attn, b_attn, W_O, b_O):
    global _NC_CACHE
    res = np.asarray(res, dtype=np.float32)
    W_attn = np.asarray(W_attn, dtype=np.float32)
    b_attn = np.asarray(b_attn, dtype=np.float32)
    W_O = np.asarray(W_O, dtype=np.float32)
    b_O = np.asarray(b_O, dtype=np.float32)

    scale = 1.0 / np.sqrt(np.float32(DH))

    in_maps = []
    for core in range(N_CORES):
        b, hg = divmod(core, HG)
        sl = slice(hg * FH, (hg + 1) * FH)
        wq = W_attn[0 * D:1 * D][sl, :] * scale     # fold 1/sqrt(dh) into q
        wk = W_attn[1 * D:2 * D][sl, :]
        wv = W_attn[2 * D:3 * D][sl, :]
        bq = b_attn[0 * D:1 * D][sl] * scale
        bk = b_attn[1 * D:2 * D][sl]
        bv = b_attn[2 * D:3 * D][sl]
        wqkvT = np.ascontiguousarray(
            np.concatenate([wq, wk, wv], axis=0).T)     # [D, 768]
        bqk = np.concatenate([bq, bk]).reshape(4, 128)  # q01,q23,k01,k23
        woT = np.ascontiguousarray(W_O[:, sl].T)        # [256, D]
        resT = np.ascontiguousarray(res[b].T)           # [D, S]
        in_maps.append({
            "resT": resT, "wqkvT": wqkvT, "bqk": np.ascontiguousarray(bqk),
            "bv": np.ascontiguousarray(bv), "woT": woT,
        })

    if _NC_CACHE is None:
        _NC_CACHE = build_nc()
    nc = _NC_CACHE

    import os
    trace = os.environ.get("ATTN_TRACE", "0") == "1"
    results = run_bass_kernel_spmd(
        nc, in_maps, core_ids=list(range(N_CORES)), trace=trace)
    if trace and results.exec_time_ns is not None:
        print(f"HW exec time: {results.exec_time_ns} ns")

    out = np.zeros((B, S, D), dtype=np.float32)
    for core in range(N_CORES):
        b = core // HG
        out[b] += results.results[core]["part"]
    out += b_O
    return out


# revision 17
# speedup vs baseline: 25.0675x; 25.0675x over previous
"""Causal multi-head attention block (dense transformer) on 8 Trainium2 cores.

Reference computation (fp32):
    qkv = res @ W_attn.T + b_attn            # [B,S,3D]
    q,k,v -> heads [B,H,S,Dh];  scores = q k^T / sqrt(Dh), causal softmax
    inter = probs @ v -> [B,S,D];  out = inter @ W_O.T + b_O

Sharding: core (b, hg) handles batch b and head-group hg (4 heads).
Each core computes its 4 heads' attention and a partial output
projection over its 256 inter dims; the host sums the 4 partials per
batch and adds b_O.

Device dataflow (per core) keeps activations feature-on-partition
("T layout") so every matmul contraction is on the partition dim:
    resT [D,S]  --W_qkv-->  qT,kT [256,S] and v [S,256] (natural)
    S^T tile = kT_h.T-slices @ qT_h   (K=Dh=64, packed 2 heads via
    tile_position row groups)
    P^T = exp(S^T) (ACT, scale folded into q), causal handled by
    skipping blocks above the diagonal, column-sliced partial tiles,
    and a 0/1 triangular mask multiply on diagonal 128x128 blocks.
    PV: lhsT = [v_h | ones64] so PSUM rows 0-63 get inter^T_h and rows
    64-127 get the softmax denominator; normalize with DVE reciprocal +
    multiply straight out of PSUM.
    out_partial = interT.T @ W_O^T-slice  -> DMA out.

All matmul operands are float32r (verified ~1.6e-4 rel err, full PE rate).
"""

import numpy as np

import concourse.bass as bass
import concourse.mybir as mybir
import concourse.tile as tile
from concourse.bass_utils import run_bass_kernel_spmd

F32 = mybir.dt.float32
F32R = mybir.dt.float32r
AF = mybir.ActivationFunctionType

# Problem constants (hardcoded per contract)
B = 2
S = 2048
D = 1024
H = 16
DH = 64
N_CORES = 8
HG = 4            # head groups (cores per batch)
NH = H // HG      # heads per core = 4
FH = NH * DH      # features per core per q/k/v = 256
DT = D // 128     # d-model tiles = 8
NT = S // 128     # sequence j-tiles = 16
SC = 512          # i-chunk width
NC_CHUNK = S // SC  # 4


def _split_multi_waits(nc, max_waits=1):
    """walrus in this container rejects >1 sync-wait per instruction; hoist
    extras onto preceding NoOps on the same engine (queue order preserves
    semantics)."""
    for fn in nc.m.functions:
        for blk in fn.blocks:
            insts = list(blk.instructions)
            out = []
            changed = False
            for inst in insts:
                si = inst.sync_info
                waits = list(si.on_wait) if (si is not None and si.on_wait) else []
                if len(waits) > max_waits:
                    extra, keep = waits[:-max_waits], waits[-max_waits:]
                    for j, w in enumerate(extra):
                        out.append(mybir.InstNoOp(
                            name=f"{inst.name}-ws{j}", ins=[], outs=[],
                            engine=inst.engine,
                            sync_info=mybir.SyncInfo(on_wait=[w], on_update=[]),
                        ))
                    inst.sync_info = mybir.SyncInfo(
                        on_wait=keep, on_update=list(si.on_update))
                    changed = True
                out.append(inst)
            if changed:
                blk.instructions = out


def build_nc(niter=1, io_mode="external", unroll=1):
    import contextlib
    nc = bass.Bass()

    if io_mode == "external":
        resT_d = nc.dram_tensor("resT", [D, S], F32R, kind="ExternalInput")
        wqkv_d = nc.dram_tensor("wqkvT", [D, 3 * FH], F32R, kind="ExternalInput")
        bqk_d = nc.dram_tensor("bqk", [4, 128], F32, kind="ExternalInput")
        bv_d = nc.dram_tensor("bv", [FH], F32, kind="ExternalInput")
        woT_d = nc.dram_tensor("woT", [FH, D], F32R, kind="ExternalInput")
        part_d = nc.dram_tensor("part", [S, D], F32, kind="ExternalOutput")
    else:
        # timing-only build: no big host<->device transfers
        resT_d = nc.dram_tensor("resT", [D, S], F32R)
        wqkv_d = nc.dram_tensor("wqkvT", [D, 3 * FH], F32R)
        bqk_d = nc.dram_tensor("bqk", [4, 128], F32)
        bv_d = nc.dram_tensor("bv", [FH], F32)
        woT_d = nc.dram_tensor("woT", [FH, D], F32R)
        part_d = nc.dram_tensor("part", [S, D], F32)
        tick_d = nc.dram_tensor("tick", [128, 1], F32, kind="ExternalInput")
        tock_d = nc.dram_tensor("tock", [128, 1], F32, kind="ExternalOutput")

    with tile.TileContext(nc) as tc:
        with (
            tc.tile_pool(name="persist", bufs=1) as persist,
            tc.tile_pool(name="work", bufs=3) as work,
            tc.tile_pool(name="expp", bufs=1) as expp,
            tc.tile_pool(name="rdenp", bufs=2) as rdenp,
            tc.tile_pool(name="outp", bufs=2) as outp,
            tc.tile_pool(name="ps_mm", bufs=3, space="PSUM") as ps_mm,
        ):
          with (tc.For_i(0, niter, 1) if niter > 1
                else contextlib.nullcontext()):
           for _rep in range(unroll):
            if io_mode != "external":
                tick_sb = persist.tile([128, 1], F32, tag="tick")
                nc.sync.dma_start(tick_sb, tick_d[:, :])
                nc.sync.dma_start(tock_d[:, :], tick_sb)
            # ---- constant / persistent loads ----
            w_sb = persist.tile([128, DT, 3 * FH], F32R, tag="w")
            nc.sync.dma_start(
                w_sb, wqkv_d[:, :].rearrange("(t p) f -> p t f", p=128))
            woT_sb = persist.tile([128, 2, D], F32R, tag="wo")
            nc.sync.dma_start(
                woT_sb, woT_d[:, :].rearrange("(t p) f -> p t f", p=128))
            bias_sb = persist.tile([128, 4], F32, tag="bqk")
            nc.sync.dma_start(
                bias_sb, bqk_d[:, :].rearrange("f p -> p f"))
            vb_sb = persist.tile([128, FH], F32, tag="bv")
            bv_ap = bv_d[:]
            nc.gpsimd.dma_start(
                vb_sb,
                bass.AP(tensor=bv_ap.tensor, offset=bv_ap.offset,
                        ap=[[0, 128], [1, FH]]))
            # 0/1 upper-triangular (keep i>=j) mask for diagonal blocks
            tri01 = persist.tile([128, 128], F32, tag="tri")
            nc.gpsimd.memset(tri01, 0.0)
            nc.gpsimd.affine_select(
                out=tri01, in_=tri01,
                compare_op=mybir.AluOpType.is_gt,
                fill=1.0, base=0, pattern=[[-1, 128]], channel_multiplier=1)

            resT_sb = []
            for dt_i in range(DT):
                t = persist.tile([128, S], F32R, tag=f"resT{dt_i}")
                nc.sync.dma_start(t, resT_d[128 * dt_i:128 * (dt_i + 1), :])
                resT_sb.append(t)

            # ---- phase A: qkv projections ----
            # qk tiles: ft in {0:q01, 1:q23, 2:k01, 3:k23} x chunk c -> [128, SC]
            qk_sb = {}
            v_sb = []
            for c in range(NC_CHUNK):
                for ft in range(4):
                    qk = persist.tile([128, SC], F32R, tag=f"qk{ft}_{c}")
                    ps = ps_mm.tile([128, SC], F32, tag="mm", name=f"psqk{ft}_{c}", padded_shape=[128, SC])
                    wcol = 128 * ft if ft < 2 else FH + 128 * (ft - 2)
                    for dt_i in range(DT):
                        nc.tensor.matmul(
                            ps, w_sb[:, dt_i, wcol:wcol + 128],
                            resT_sb[dt_i][:, SC * c:SC * (c + 1)],
                            start=(dt_i == 0), stop=(dt_i == DT - 1))
                    nc.scalar.activation(qk, ps, AF.Identity,
                                         bias=bias_sb[:, ft:ft + 1], scale=1.0)
                    qk_sb[(ft, c)] = qk
                # v s-tiles for this chunk: st = 4c..4c+3, layout per head
                # [v_h(64) | ones(64)] -> [128, NH*128]
                for st in range(4 * c, 4 * c + 4):
                    vt = persist.tile([128, NH, 128], F32R, tag=f"v{st}")
                    for h in range(NH):
                        nc.vector.memset(vt[:, h, DH:128].bitcast(F32), 1.0)
                    psv = ps_mm.tile([128, FH], F32, tag="mm", name=f"psv{st}", padded_shape=[128, SC])
                    for dt_i in range(DT):
                        nc.tensor.matmul(
                            psv, resT_sb[dt_i][:, 128 * st:128 * (st + 1)],
                            w_sb[:, dt_i, 2 * FH:3 * FH],
                            start=(dt_i == 0), stop=(dt_i == DT - 1))
                    nc.vector.tensor_tensor(
                        vt[:, :, 0:DH],
                        psv.rearrange("p (h d) -> p h d", d=DH),
                        vb_sb.rearrange("p (h d) -> p h d", d=DH),
                        mybir.AluOpType.add)
                    v_sb.append(vt)

            # ---- phase B: attention per chunk ----
            interT = {}  # (kt, c) -> [128, SC] f32r
            with (
                tc.tile_pool(name="ps_pv", bufs=1, space="PSUM") as ps_pv,
            ):
                for c in range(NC_CHUNK):
                    pv = ps_pv.tile([128, NH, SC], F32, tag="pv")
                    for h in range(NH):
                        kt_idx, par = divmod(h, 2)
                        base = 64 * par
                        ntile = 4 * c + 4
                        expP = expp.tile([128, 4, SC], F32R, tag="expP")
                        for t in range(ntile):
                            partial = t >= 4 * c
                            dpos = t - 4 * c
                            lo = 128 * dpos if partial else 0
                            sps = ps_mm.tile([128, SC], F32, tag="mm",
                                             name=f"sps{c}_{h}_{t}")
                            kt_tile = qk_sb[(2 + kt_idx, t // 4)]
                            q_tile = qk_sb[(kt_idx, c)]
                            nc.tensor.matmul(
                                sps[:, lo:SC],
                                kt_tile[base:base + 64,
                                        128 * (t % 4):128 * (t % 4) + 128],
                                q_tile[base:base + 64, lo:SC],
                                start=True, stop=True,
                                tile_position=(base, 0))
                            if partial:
                                nc.scalar.activation(
                                    expP[:, dpos, lo:SC], sps[:, lo:SC], AF.Exp)
                            else:
                                expF = work.tile([128, SC], F32R, tag="expF")
                                nc.scalar.activation(expF, sps, AF.Exp)
                                nc.tensor.matmul(
                                    pv[:, h, :], v_sb[t][:, h, :], expF,
                                    start=(t == 0), stop=False)
                        # mask diagonal 128-blocks of the 4 partial tiles
                        for dpos in range(4):
                            nc.vector.tensor_tensor(
                                expP[:, dpos, 128 * dpos:128 * dpos + 128],
                                expP[:, dpos, 128 * dpos:128 * dpos + 128],
                                tri01, mybir.AluOpType.mult)
                        for dpos in range(4):
                            t = 4 * c + dpos
                            lo = 128 * dpos
                            nc.tensor.matmul(
                                pv[:, h, lo:SC], v_sb[t][:, h, :],
                                expP[:, dpos, lo:SC],
                                start=(t == 0), stop=(dpos == 3))
                    # normalize: rows 64-127 of each pv bank hold the denom
                    for h in range(NH):
                        kt_idx, par = divmod(h, 2)
                        key = (kt_idx, c)
                        if key not in interT:
                            interT[key] = persist.tile(
                                [128, SC], F32R, tag=f"it{kt_idx}_{c}",
                                name=f"interT{kt_idx}_{c}")
                        rden = rdenp.tile([128, SC], F32, tag="rden",
                                          name=f"rden{c}_{h}")
                        nc.vector.reciprocal(rden[64:128, :], pv[64:128, h, :])
                        nc.vector.tensor_tensor(
                            interT[key][64 * par:64 * par + 64, :],
                            pv[0:64, h, :], rden[64:128, :],
                            mybir.AluOpType.mult)

            # ---- phase C: output projection ----
            if True:
                for st in range(NT):
                    ot = outp.tile([128, D], F32, tag="ot")
                    for dc in range(2):
                        po = ps_mm.tile([128, 512], F32, tag="mm",
                                        name=f"po{st}_{dc}")
                        for kt in range(2):
                            nc.tensor.matmul(
                                po,
                                interT[(kt, st // 4)][:, 128 * (st % 4):
                                                      128 * (st % 4) + 128],
                                woT_sb[:, kt, 512 * dc:512 * (dc + 1)],
                                start=(kt == 0), stop=(kt == 1))
                        nc.vector.tensor_copy(ot[:, 512 * dc:512 * (dc + 1)], po)
                    nc.sync.dma_start(part_d[128 * st:128 * (st + 1), :], ot)

    _split_multi_waits(nc)
    return nc


_NC_CACHE = None


def kernel(res, W_attn, b_attn, W_O, b_O):
    global _NC_CACHE
    res = np.asarray(res, dtype=np.float32)
    W_attn = np.asarray(W_attn, dtype=np.float32)
    b_attn = np.asarray(b_attn, dtype=np.float32)
    W_O = np.asarray(W_O, dtype=np.float32)
    b_O = np.asarray(b_O, dtype=np.float32)

    scale = 1.0 / np.sqrt(np.float32(DH))

    in_maps = []
    for core in range(N_CORES):
        b, hg = divmod(core, HG)
        sl = slice(hg * FH, (hg + 1) * FH)
        wq = W_attn[0 * D:1 * D][sl, :] * scale     # fold 1/sqrt(dh) into q
        wk = W_attn[1 * D:2 * D][sl, :]
        wv = W_attn[2 * D:3 * D][sl, :]
        bq = b_attn[0 * D:1 * D][sl] * scale
        bk = b_attn[1 * D:2 * D][sl]
        bv = b_attn[2 * D:3 * D][sl]
        wqkvT = np.ascontiguousarray(
            np.concatenate([wq, wk, wv], axis=0).T)     # [D, 768]
        bqk = np.concatenate([bq, bk]).reshape(4, 128)  # q01,q23,k01,k23
        woT = np.ascontiguousarray(W_O[:, sl].T)        # [256, D]
        resT = np.ascontiguousarray(res[b].T)           # [D, S]
        in_maps.append({
            "resT": resT, "wqkvT": wqkvT, "bqk": np.ascontiguousarray(bqk),
            "bv": np.ascontiguousarray(bv), "woT": woT,
        })

    if _NC_CACHE is None:
        _NC_CACHE = build_nc()
    nc = _NC_CACHE

    import os
    trace = os.environ.get("ATTN_TRACE", "0") == "1"
    results = run_bass_kernel_spmd(
        nc, in_maps, core_ids=list(range(N_CORES)), trace=trace)
    if trace and results.exec_time_ns is not None:
        print(f"HW exec time: {results.exec_time_ns} ns")

    out = np.zeros((B, S, D), dtype=np.float32)
    for core in range(N_CORES):
        b = core // HG
        out[b] += results.results[core]["part"]
    out += b_O
    return out
